# revision 33
# baseline (speedup 1.0000x reference)
"""AttentionSuper (AutoFormer 2D rel-pos attention) Trainium2 Bass kernel.

B=64,N=197,C=640,H=10,D=64 ; data-parallel over batch: 8 batches/core x 8 cores.

v3 design: no DMA gathers and no partition-shifted engine copies (both are
illegal/slow). The rel-pos-K gather G is computed directly from q by 28 small
matmuls per batch whose lhsT is a free-dim-shifted slice of the rel-pos table
(one per 14-query block / column class), written to PSUM and copied to
32-aligned partition rows of the QG stack. The score matmul contract-stacks
[kT | EF2v | 0 | EF2h] x [qT | G_v | ? | G_h] (contract 110). The cls-KEY
bias folds into the k tiles as a per-batch column bias. Block sums + rowsum
come from one matmul pair against a constant [cls|E2v|E2h|ones] lhsT; the
rel-pos-V coefficient matrix C2 is built by shifted one-hot scatter matmuls
from the block sums; att@v + rel-pos-V accumulate in one PSUM group and
normalization is a single multiply against a PE-broadcast reciprocal.
Final bias add + cls-row rpv fix happen on the host.
"""

import os
import numpy as np
import ml_dtypes

B, N, C = 64, 197, 640
H, D = 10, 64
MRP = 14
TABLE = 30
NB = 8          # batches per core
NCORES = 8
NT = NB * N     # 1576 tokens per core
NP = 200        # per-batch q block width (197 + 3 pad)
KQ = 110        # score contract: 64 q/k + 14 G_v + (18 zero) + 14 G_h
C2R = 65        # C2 rows: 30 v + 2 zero + 30 h + 2 zero + 1 cls
SCALE = D ** -0.5


def _consts():
    r = np.arange(N - 1)
    kr = r // 14
    kc = r % 14
    # E2x (197,33): bst rows: 0 cls, 1..14 v-blocks, 16..29 h-blocks, 32 ones
    E2 = np.zeros((N, 33), np.float32)
    E2[0, 0] = 1.0
    E2[1:, :][np.arange(N - 1), 1 + kr] = 1.0
    E2[1:, :][np.arange(N - 1), 16 + kc] = 1.0
    E2[:, 32] = 1.0
    # EF2x (46,197): KE rows 64:110. rows 0:14 v-blocks, 14:32 zero, 32:46 h
    EF2 = np.zeros((46, N), np.float32)
    EF2[:, 1:][kr, np.arange(N - 1)] = 1.0
    EF2[:, 1:][32 + kc, np.arange(N - 1)] = 1.0
    # SCAT (33, 92): cols 0:46 v-scatter, 46:92 h-scatter one-hots
    # (32-wide slices so the scatter matmul fully covers C2 rows incl zeros)
    SCAT = np.zeros((33, 92), np.float32)
    for t in range(14):
        SCAT[1 + t, 15 + t] = 1.0       # v: C2[j,blk r]=bst[1+(r+j-15)]
        SCAT[16 + t, 46 + 15 + t] = 1.0  # h: C2[32+j,cls c]=bst[16+(c+j-15)]
    return E2, EF2, SCAT


def _build_nc():
    import concourse.bass as bass  # noqa: F401
    import concourse.mybir as mybir
    from concourse import bacc
    from concourse.tile import TileContext

    f32 = mybir.dt.float32
    bf16 = mybir.dt.bfloat16
    EXP = mybir.ActivationFunctionType.Exp
    MULT = mybir.AluOpType.mult

    nc = bacc.Bacc("TRN2", target_bir_lowering=False, debug=False,
                   num_devices=NCORES)

    xT_d = nc.dram_tensor("xT", [C, NT], bf16, kind="ExternalInput")
    wqkvT_d = nc.dram_tensor("wqkvT", [C, 3 * C], bf16, kind="ExternalInput")
    wprojT_d = nc.dram_tensor("wprojT", [C, C], bf16, kind="ExternalInput")
    rpkT1_d = nc.dram_tensor("rpkT1", [D, 2 * TABLE], bf16, kind="ExternalInput")
    rpkcls_d = nc.dram_tensor("rpkcls", [D, 1], f32, kind="ExternalInput")
    rpv2_d = nc.dram_tensor("rpv2", [C2R, D], bf16, kind="ExternalInput")
    EF2rep_d = nc.dram_tensor("EF2rep", [46, NT], bf16, kind="ExternalInput")
    E2ones_d = nc.dram_tensor("E2ones", [N, 33], bf16, kind="ExternalInput")
    scat_d = nc.dram_tensor("scat", [33, 92], bf16, kind="ExternalInput")
    sel2_d = nc.dram_tensor("sel2", [2, 128], bf16, kind="ExternalInput")
    yT_d = nc.dram_tensor("yT", [C, NT], bf16, kind="ExternalOutput")

    from contextlib import ExitStack
    with TileContext(nc) as tc, ExitStack() as es:
        es.enter_context(nc.allow_low_precision(reason="bf16 attention kernel"))
        cp = es.enter_context(tc.tile_pool(name="const", bufs=1))
        bsp = es.enter_context(tc.tile_pool(name="bstb", bufs=2))
        c2p = es.enter_context(tc.tile_pool(name="c2", bufs=2))
        rsp = es.enter_context(tc.tile_pool(name="rs", bufs=2))
        atp = es.enter_context(tc.tile_pool(name="at", bufs=2))
        ysp = es.enter_context(tc.tile_pool(name="ys", bufs=2))
        # PSUM: bank-granular slots; exactly 8 banks
        pbig = es.enter_context(tc.tile_pool(name="pbig", bufs=2, space="PSUM"))
        pst = es.enter_context(tc.tile_pool(name="pst", bufs=2, space="PSUM"))
        psc = es.enter_context(tc.tile_pool(name="psc", bufs=4, space="PSUM"))

        # ---- loads ----
        wq = [cp.tile([128, 3 * C], bf16, tag=f"wq{c}", name=f"wq{c}") for c in range(5)]
        wp = [cp.tile([128, C], bf16, tag=f"wp{c}", name=f"wp{c}") for c in range(5)]
        xT = [cp.tile([128, NT], bf16, tag=f"xT{c}", name=f"xT{c}") for c in range(5)]
        dmae = [nc.sync, nc.scalar, nc.gpsimd]
        for c in range(5):
            for half in range(2):
                p0, p1 = 64 * half, 64 * (half + 1)
                dmae[(2 * c + half) % 3].dma_start(
                    out=xT[c][p0:p1, :], in_=xT_d[128 * c + p0:128 * c + p1, :])
                dmae[(2 * c + half + 1) % 3].dma_start(
                    out=wq[c][p0:p1, :], in_=wqkvT_d[128 * c + p0:128 * c + p1, :])
            dmae[(c + 2) % 3].dma_start(out=wp[c][:], in_=wprojT_d[128 * c:128 * (c + 1), :])
        rpkT1 = cp.tile([D, 2 * TABLE], bf16)
        rpkclsb = cp.tile([D, 1], f32)
        rpv2 = cp.tile([C2R, D], bf16)
        scat = cp.tile([33, 92], bf16)
        sel2 = cp.tile([2, 128], bf16)
        e2o = [cp.tile([128, 33], bf16, tag="e2o0", name="e2o0"),
               cp.tile([69, 33], bf16, tag="e2o1", name="e2o1")]
        nc.scalar.dma_start(out=rpkT1[:], in_=rpkT1_d[:])
        nc.scalar.dma_start(out=rpkclsb[:], in_=rpkcls_d[:])
        nc.scalar.dma_start(out=rpv2[:], in_=rpv2_d[:])
        nc.scalar.dma_start(out=scat[:], in_=scat_d[:])
        nc.scalar.dma_start(out=sel2[:], in_=sel2_d[:])
        nc.scalar.dma_start(out=e2o[0][:], in_=E2ones_d[0:128, :])
        nc.scalar.dma_start(out=e2o[1][:], in_=E2ones_d[128:197, :])

        # KE_h [110, NT]: 0:64 k feats, 64:110 EF2x (zeros rows 78:96)
        KE = [cp.tile([KQ, NT], bf16, tag=f"ke{h}", name=f"ke{h}") for h in range(H)]
        for h in range(H):
            nc.gpsimd.dma_start(out=KE[h][64:KQ, :], in_=EF2rep_d[:])
        # QGall [110, NB*2000]: per batch: 0:64 q per head block, 64:78 G_v,
        # 96:110 G_h. One tile so G generation can span batches in one matmul.
        QGall = cp.tile([KQ, NB * H * NP], bf16, tag="qgall", name="qgall")
        QG = [QGall[:, b * H * NP:(b + 1) * H * NP] for b in range(NB)]
        for b in range(NB):
            qq = QG[b].rearrange("p (a w) -> p a w", a=H)
            nc.gpsimd.memset(qq[0:KQ, :, N:NP], 0.0)   # q+G pad cols
            nc.gpsimd.memset(qq[64:KQ, :, 0:1], 0.0)   # G cls col (bias 0)
            nc.gpsimd.memset(QG[b][64:96, :], 0.0)     # zero unused G rows 78:96
        vt0 = [cp.tile([128, C], bf16, tag=f"v0{b}", name=f"v0{b}") for b in range(NB)]
        vt1 = [cp.tile([69, C], bf16, tag=f"v1{b}", name=f"v1{b}") for b in range(NB)]
        outT = [cp.tile([128, NT], bf16, tag=f"oT{m}", name=f"oT{m}") for m in range(5)]

        # ---- phase A: projections ----
        kchunks = [(0, 512), (512, 512), (1024, 512), (1536, 40)]
        for m in range(5):
            for (o0, on) in kchunks:
                acc = pbig.tile([128, on], f32, tag="big", name="acck")
                for c in range(5):
                    nc.tensor.matmul(acc[:], wq[c][:, C + 128 * m:C + 128 * (m + 1)],
                                     xT[c][:, o0:o0 + on],
                                     start=(c == 0), stop=(c == 4))
                nc.scalar.copy(out=KE[2 * m][0:64, o0:o0 + on], in_=acc[0:64, :])
                nc.scalar.copy(out=KE[2 * m + 1][0:64, o0:o0 + on], in_=acc[64:128, :])
        for h in range(H):  # cls-KEY rel-pos bias folded into k columns
            nc.vector.tensor_scalar_add(out=KE[h][0:64, 0:NT:N],
                                        in0=KE[h][0:64, 0:NT:N],
                                        scalar1=rpkclsb[:, 0:1])
        for m in range(5):
            for b in range(NB):
                t0 = b * N
                acc = pst.tile([128, N], f32, tag="st", name="accq")
                for c in range(5):
                    nc.tensor.matmul(acc[:], wq[c][:, 128 * m:128 * (m + 1)],
                                     xT[c][:, t0:t0 + N],
                                     start=(c == 0), stop=(c == 4))
                h0, h1 = 2 * m, 2 * m + 1
                nc.vector.tensor_copy(out=QG[b][0:64, h0 * NP:h0 * NP + N],
                                      in_=acc[0:64, :])
                nc.vector.tensor_copy(out=QG[b][0:64, h1 * NP:h1 * NP + N],
                                      in_=acc[64:128, :])
        for b in range(NB):
            t0 = b * N
            toksl = [(t0, vt0[b], 128), (t0 + 128, vt1[b], 69)]
            for si, (ts, vdst, tn) in enumerate(toksl):
                for (o0, on) in [(0, 512), (512, 128)]:
                    acc = psc.tile([tn, on], f32, tag="sc", name="accv")
                    for c in range(5):
                        nc.tensor.matmul(acc[:], xT[c][:, ts:ts + tn],
                                         wq[c][:, 2 * C + o0:2 * C + o0 + on],
                                         start=(c == 0), stop=(c == 4))
                    eng = nc.vector if (b + si) % 2 == 0 else nc.scalar
                    if eng is nc.scalar:
                        eng.copy(out=vdst[:, o0:o0 + on], in_=acc[:])
                    else:
                        eng.tensor_copy(out=vdst[:, o0:o0 + on], in_=acc[:])

        # ---- batch loop ----
        saved = [None] * NB

        def pass3(b, C2t):
            at0, at1, rcp2 = saved[b]
            t0 = b * N
            rbs = rsp.tile([128, (H // 2) * NP], bf16, tag="rbs", name="rbs")
            for (i0, iw) in [(0, 2), (2, 2), (4, 1)]:
                rb_ps = psc.tile([128, iw * NP], f32, tag="sc", name="rbps")
                nc.tensor.matmul(rb_ps[:], sel2[:],
                                 rcp2[:, i0 * NP:(i0 + iw) * NP],
                                 start=True, stop=True)
                nc.vector.tensor_copy(out=rbs[:, i0 * NP:(i0 + iw) * NP],
                                      in_=rb_ps[:])
            for i in range(H // 2):
                for hh in range(2):
                    h = 2 * i + hh
                    av = pst.tile([64, NP], f32, tag="st", name="avps")
                    nc.tensor.matmul(av[:], vt0[b][:, 64 * h:64 * h + 64],
                                     at0[:, h * NP:h * NP + NP],
                                     start=True, stop=False)
                    nc.tensor.matmul(av[:], vt1[b][:, 64 * h:64 * h + 64],
                                     at1[:, h * NP:h * NP + NP],
                                     start=False, stop=False)
                    nc.tensor.matmul(av[:], rpv2[:],
                                     C2t[:, h * NP:h * NP + NP],
                                     start=False, stop=True)
                    dst = outT[i][hh * 64:hh * 64 + 64, t0:t0 + N]
                    nc.vector.tensor_tensor(
                        out=dst, in0=av[:, 0:N],
                        in1=rbs[hh * 64:hh * 64 + 64, i * NP:i * NP + N],
                        op=MULT)

        # G generation: 3-batch groups, one matmul per (group, part, block)
        ggroups = [(0, 3), (3, 3), (6, 2)]

        def gen_G_chunk(g, idx):
            b0, gn = ggroups[g]
            Qg = QGall[:, b0 * H * NP:(b0 + gn) * H * NP].rearrange(
                "p (bb a w) -> p bb a w", bb=gn, a=H)
            part, prow = ((0, 64), (1, 96))[idx // 14]
            r = idx % 14
            if part == 0:
                lh = rpkT1[:, 15 - r:29 - r]
                rh = Qg[0:64, :, :, 1 + 14 * r:15 + 14 * r]
                dst = Qg[prow:prow + 14, :, :, 1 + 14 * r:15 + 14 * r]
            else:
                lh = rpkT1[:, 45 - r:59 - r]
                rh = Qg[0:64, :, :, 1 + r:185 + r:14]
                dst = Qg[prow:prow + 14, :, :, 1 + r:185 + r:14]
            ps = psc.tile([14, gn * 140], f32, tag="sc", name="gps")
            nc.tensor.matmul(ps[:], lh, rh, start=True, stop=True)
            pv = ps[:].rearrange("p (bb a w) -> p bb a w", bb=gn, a=H)
            nc.vector.tensor_copy(out=dst, in_=pv)

        # C2: per batch, chunked over 14-block groups
        gch = [(0, 3), (3, 3), (6, 3), (9, 3), (12, 2)]

        def gen_C2_chunk(bstb, C2t, idx):
            Cq = C2t[:].rearrange("p (a w) -> p a w", a=H)
            bq = bstb[:].rearrange("p (a w) -> p a w", a=H)
            part, prow = ((0, 0), (1, 32))[idx // 5]
            r0, rn = gch[idx % 5]
            ps = psc.tile([32, rn * 140], f32, tag="sc", name="c2ps")
            for j in range(rn):
                r = r0 + j
                if part == 0:
                    lh = scat[:, r:r + 32]
                    rh = bq[0:33, :, 1 + 14 * r:15 + 14 * r]
                else:
                    lh = scat[:, 46 + r:46 + r + 32]
                    rh = bq[0:33, :, 1 + r:185 + r:14]
                nc.tensor.matmul(ps[:, 140 * j:140 * (j + 1)], lh, rh,
                                 start=True, stop=True)
            pv = ps[:].rearrange("p (j a w) -> p j a w", j=rn, a=H)
            if part == 0:
                dst = Cq[prow:prow + 32, :, 1 + 14 * r0:1 + 14 * (r0 + rn)]
                dst = dst.rearrange("p a (j w) -> p j a w", j=rn)
                nc.scalar.copy(out=dst, in_=pv)
            else:
                for j in range(rn):
                    r = r0 + j
                    nc.scalar.copy(
                        out=Cq[prow:prow + 32, :, 1 + r:185 + r:14],
                        in_=pv[:, j, :, :])

        def fin_C2(bstb, C2t):
            Cq = C2t[:].rearrange("p (a w) -> p a w", a=H)
            bq = bstb[:].rearrange("p (a w) -> p a w", a=H)
            nc.gpsimd.memset(Cq[:, :, 0:1], 0.0)
            nc.gpsimd.memset(Cq[:, :, N:NP], 0.0)
            nc.vector.tensor_copy(out=Cq[64:65, :, 1:197], in_=bq[0:1, :, 1:197])

        from collections import deque
        pend_G = deque()
        for i in range(28):
            gen_G_chunk(0, i)

        prev_bstb = None
        for b in range(NB):
            t0 = b * N
            if b in (0, 3, 6) and b // 3 + 1 < len(ggroups):
                pend_G.extend((b // 3 + 1, i) for i in range(28))
            C2p = None
            if b > 0:
                C2p = c2p.tile([C2R, H * NP], bf16, tag="c2", name="c2")

            bstb = bsp.tile([33, H * NP], bf16, tag="bstb", name="bstb")
            rsb = rsp.tile([H, NP], bf16, tag="rsb", name="rsb")
            at0 = atp.tile([128, H * NP], bf16, tag="at0", name="at0")
            at1 = atp.tile([69, H * NP], bf16, tag="at1", name="at1")

            def bst_pair(i):
                bst = psc.tile([33, 2 * NP], f32, tag="sc", name="bstps")
                nc.tensor.matmul(bst[:], e2o[0][:],
                                 at0[:, 2 * i * NP:(2 * i + 2) * NP],
                                 start=True, stop=False)
                nc.tensor.matmul(bst[:], e2o[1][:],
                                 at1[:, 2 * i * NP:(2 * i + 2) * NP],
                                 start=False, stop=True)
                nc.vector.tensor_copy(
                    out=bstb[:, 2 * i * NP:(2 * i + 2) * NP], in_=bst[:])

            for h in range(H):
                for s, (k0, kn, atall) in enumerate(
                        [(0, 128, at0), (128, 69, at1)]):
                    st = pst.tile([kn, NP], f32, tag="st", name="stps")
                    nc.tensor.matmul(st[:], KE[h][:, t0 + k0:t0 + k0 + kn],
                                     QG[b][:, h * NP:h * NP + NP],
                                     start=True, stop=True)
                    nc.scalar.activation(out=atall[:, h * NP:h * NP + NP],
                                         in_=st[:], func=EXP, scale=SCALE)
                if h % 2 == 1 and h > 1:
                    bst_pair(h // 2 - 1)
                if b > 0:
                    gen_C2_chunk(prev_bstb, C2p, h)
                if pend_G:
                    gen_G_chunk(*pend_G.popleft())
                if pend_G and h >= 5:
                    gen_G_chunk(*pend_G.popleft())
            bst_pair(4)

            # rowsums (bstb row 32) -> [10, NP] via DMA (partition regroup)
            nc.sync.dma_start(out=rsb[:],
                              in_=bstb[32:33, :].rearrange("p (a w) -> p a w", a=H))
            rcpb = rsp.tile([H, NP], bf16, tag="rcpb", name="rcpb")
            nc.vector.reciprocal(out=rcpb[:], in_=rsb[:])
            rcp2 = rsp.tile([2, (H // 2) * NP], bf16, tag="rcp2", name="rcp2")
            nc.sync.dma_start(out=rcp2[0:1, :].rearrange("p (i w) -> p i w", i=H // 2),
                              in_=rcpb[0:H:2, :])
            nc.sync.dma_start(out=rcp2[1:2, :].rearrange("p (i w) -> p i w", i=H // 2),
                              in_=rcpb[1:H:2, :])
            saved[b] = (at0, at1, rcp2)

            if b > 0:
                fin_C2(prev_bstb, C2p)
                pass3(b - 1, C2p)
            prev_bstb = bstb

        # tail: C2 + pass3 for the last batch
        C2p = c2p.tile([C2R, H * NP], bf16, tag="c2", name="c2")
        for i in range(10):
            gen_C2_chunk(prev_bstb, C2p, i)
        fin_C2(prev_bstb, C2p)
        pass3(NB - 1, C2p)

        # ---- final projection ----
        for m in range(5):
            for (o0, on) in kchunks:
                acc = (pbig.tile([128, on], f32, tag="big", name="accy")
                       if m % 2 == 0 else
                       pst.tile([128, on], f32, tag="st", name="accy"))
                for c in range(5):
                    nc.tensor.matmul(acc[:], wp[c][:, 128 * m:128 * (m + 1)],
                                     outT[c][:, o0:o0 + on],
                                     start=(c == 0), stop=(c == 4))
                ysb = ysp.tile([128, on], bf16, tag="ysb", name="ysb")
                if m % 2 == 0:
                    nc.vector.tensor_copy(out=ysb[:], in_=acc[:])
                else:
                    nc.scalar.copy(out=ysb[:], in_=acc[:])
                nc.sync.dma_start(out=yT_d[128 * m:128 * (m + 1), o0:o0 + on],
                                  in_=ysb[:])

    nc.compile()
    return nc


_NC_CACHE = None


def kernel(x, w_qkv, w_proj, b_proj, rpk_v, rpk_h, rpv_v, rpv_h):
    global _NC_CACHE
    from concourse.bass_utils import run_bass_kernel_spmd

    if _NC_CACHE is None:
        _NC_CACHE = _build_nc()
    nc = _NC_CACHE

    E2, EF2, SCAT = _consts()
    f32 = np.float32
    wqkvT = np.ascontiguousarray(np.asarray(w_qkv, f32).T).astype(ml_dtypes.bfloat16)
    wprojT = np.ascontiguousarray(np.asarray(w_proj, f32).T).astype(ml_dtypes.bfloat16)
    rpkT1 = np.concatenate([np.asarray(rpk_v, f32).T,
                            np.asarray(rpk_h, f32).T], axis=1).astype(ml_dtypes.bfloat16)
    rpkcls = (np.asarray(rpk_v[0], f32) + np.asarray(rpk_h[0], f32)).reshape(D, 1)
    rpv2 = np.zeros((C2R, D), f32)
    rpv2[0:30] = np.asarray(rpv_v, f32)
    rpv2[32:62] = np.asarray(rpv_h, f32)
    rpv2[64] = np.asarray(rpv_v[0], f32) + np.asarray(rpv_h[0], f32)
    rpv2 = rpv2.astype(ml_dtypes.bfloat16)
    EF2rep = np.tile(EF2, (1, NB)).astype(ml_dtypes.bfloat16)
    E2ones = E2.astype(ml_dtypes.bfloat16)
    scat = SCAT.astype(ml_dtypes.bfloat16)
    sel2 = np.zeros((2, 128), f32)
    sel2[0, 0:64] = 1.0
    sel2[1, 64:128] = 1.0
    sel2 = sel2.astype(ml_dtypes.bfloat16)

    in_maps = []
    for i in range(NCORES):
        xs = np.asarray(x[i * NB:(i + 1) * NB], f32).reshape(NT, C)
        xT = np.ascontiguousarray(xs.T).astype(ml_dtypes.bfloat16)
        in_maps.append({
            "xT": xT, "wqkvT": wqkvT, "wprojT": wprojT,
            "rpkT1": rpkT1, "rpkcls": rpkcls, "rpv2": rpv2,
            "EF2rep": EF2rep, "E2ones": E2ones, "scat": scat, "sel2": sel2,
        })

    trace = bool(os.environ.get("BASS_KERNEL_TRACE"))
    kw = {}
    if trace:
        kw = dict(trace=True, tmpdir=os.environ.get("BASS_KERNEL_TRACE_DIR") or None)
    res = run_bass_kernel_spmd(nc, in_maps, core_ids=list(range(NCORES)), **kw)
    kernel.last_result = res

    y = np.empty((B, N, C), f32)
    for i in range(NCORES):
        y[i * NB:(i + 1) * NB] = np.asarray(
            res.results[i]["yT"], dtype=f32).T.reshape(NB, N, C)
    y += np.asarray(b_proj, f32)
    # cls-row rel-pos-v correction (constant across k, softmax weights sum to 1)
    rep = np.tile((np.asarray(rpv_v[0], f32) + np.asarray(rpv_h[0], f32)), H)
    y[:, 0, :] += np.asarray(w_proj, f32) @ rep
    return y
